# revision 22
# baseline (speedup 1.0000x reference)
"""Trainium2 Bass kernel: CrossAttnBlock (16x4096x512 query, 16x77x768 cond).

Sharding: pure data-parallel over batch -- 2 batches per core on 8 cores,
no collectives.  Host-side work is layout-only (transposes / slicing / bf16
cast).

On-device dataflow per core (activations kept transposed: feature dim on
SBUF partitions, tokens on the free dim; all matmul inputs bf16, PSUM f32):
    qT = wqT-stationary matmuls over xT chunks  [512f x 512t] per chunk
    kT = wkT-stationary matmuls over condT      [512f x 154s]
    v  = condT-stationary matmuls               [77s x 512d] -> v_aug [77, 8*65]
    scoresT_h = kT_h stationary @ qT_h          [77s x 512t]  (row-packed pairs)
    e_h = exp(scoresT_h / 8)                    (no max subtraction; scores ~ +-2)
    avT_h = v_aug_h @ e_h      [65, 512t] (row 64 = softmax denominator)
    denominator rows gathered by DMA into [8, 512], DVE cast + approx-
    reciprocal, DRAM bounce + stride-0 replicate DMAs broadcast the
    reciprocals, 2 wide bf16 DVE multiplies produce norm [128, 4*CHUNK]
    yT = woT-stationary matmuls over norm column-slices; bf16 evac + DMA out

The emission order is software-pipelined across chunks so each engine's
in-order stream has its cross-engine dependencies already satisfied:
    iter i:  scores+exp(i) | dma(i+2) | Qproj(i+1) | Oproj(i-3) | av+norm(i-1)
PSUM-evacuation copies are balanced across the Scalar and Vector queues;
the whole reciprocal chain stays on the Vector queue so the Scalar queue
never head-of-line blocks on it.  Score matmul head-pairs auto-derive
tile_position (0,0)/(64,0) and run concurrently on the PE's row groups.
"""

import os
import numpy as np

MODEL_DIM = 512
COND_DIM = 768
HEAD_DIM = 64
N_HEADS = 8
B = 16
T = 4096
LK = 77
N_CORES = 8
NB = B // N_CORES          # batches per core
CHUNK = 512                # tokens per chunk
NCHUNK = T // CHUNK
NCTOT = NB * NCHUNK        # total chunks per core
KD = MODEL_DIM // 128      # 4 partition tiles of model dim
CDT = COND_DIM // 128      # 6 partition tiles of cond dim
SCALE = HEAD_DIM ** -0.5

_PROG = None               # cached compiled Bass program
LAST_RESULTS = None        # BassKernelResults of last run (for profiling)


def _build_program():
    import concourse.bass as bass  # noqa: F401
    import concourse.tile as tile
    from concourse import bacc, mybir
    from contextlib import ExitStack

    f32 = mybir.dt.float32
    bf16 = mybir.dt.bfloat16
    Exp = mybir.ActivationFunctionType.Exp

    nc = bacc.Bacc(
        "TRN2", target_bir_lowering=False, debug=False, num_devices=N_CORES
    )

    xt = nc.dram_tensor("xt", [NB, MODEL_DIM, T], bf16, kind="ExternalInput").ap()
    condt = nc.dram_tensor(
        "condt", [COND_DIM, NB * LK], bf16, kind="ExternalInput"
    ).ap()
    wqt = nc.dram_tensor("wqt", [MODEL_DIM, MODEL_DIM], bf16, kind="ExternalInput").ap()
    wkt = nc.dram_tensor("wkt", [COND_DIM, MODEL_DIM], bf16, kind="ExternalInput").ap()
    wvt = nc.dram_tensor("wvt", [COND_DIM, MODEL_DIM], bf16, kind="ExternalInput").ap()
    wot = nc.dram_tensor("wot", [MODEL_DIM, MODEL_DIM], bf16, kind="ExternalInput").ap()
    yt = nc.dram_tensor("yt", [NB, MODEL_DIM, T], bf16, kind="ExternalOutput").ap()

    with tile.TileContext(nc) as tc, ExitStack() as ctx:
        wp = ctx.enter_context(tc.tile_pool(name="wp", bufs=1))
        bp = ctx.enter_context(tc.tile_pool(name="bp", bufs=1))   # per-batch stuff
        xp = ctx.enter_context(tc.tile_pool(name="xp", bufs=3))   # x chunks
        qp = ctx.enter_context(tc.tile_pool(name="qp", bufs=3))   # qT chunks
        epool = ctx.enter_context(tc.tile_pool(name="epool", bufs=3))
        avp = ctx.enter_context(tc.tile_pool(name="avp", bufs=3))  # evac'd attnV
        rp = ctx.enter_context(tc.tile_pool(name="rp", bufs=3))    # denom rows
        rfp = ctx.enter_context(tc.tile_pool(name="rfp", bufs=3))  # 1/denom f32
        rbp = ctx.enter_context(tc.tile_pool(name="rbp", bufs=3))  # 1/denom bf16
        bcp = ctx.enter_context(tc.tile_pool(name="bcp", bufs=2))  # broadcasts
        drp = ctx.enter_context(tc.tile_pool(name="drp", bufs=3, space="DRAM"))
        npool = ctx.enter_context(tc.tile_pool(name="npool", bufs=3))
        yp = ctx.enter_context(tc.tile_pool(name="yp", bufs=2))
        pq = ctx.enter_context(tc.tile_pool(name="pq", bufs=2, space="PSUM"))
        ps = ctx.enter_context(tc.tile_pool(name="ps", bufs=2, space="PSUM"))
        pav = ctx.enter_context(tc.tile_pool(name="pav", bufs=2, space="PSUM"))
        py = ctx.enter_context(tc.tile_pool(name="py", bufs=2, space="PSUM"))

        # ---- load weights ----
        def load_rows(pool, dram_ap, n_tiles, free, tagbase, split=1):
            tiles = []
            for k in range(n_tiles):
                t_ = pool.tile([128, free], bf16, tag=f"{tagbase}{k}",
                               name=f"{tagbase}{k}")
                step = 128 // split
                for s in range(split):
                    nc.sync.dma_start(
                        out=t_[s * step:(s + 1) * step, :],
                        in_=dram_ap[k * 128 + s * step:k * 128 + (s + 1) * step, :],
                    )
                tiles.append(t_)
            return tiles

        wq_sb = load_rows(wp, wqt, KD, MODEL_DIM, "wq")

        # ---- software-pipelined chunk stages --------------------------------
        # chunk g: batch b = g // NCHUNK, token offset t0 = (g % NCHUNK)*CHUNK
        xt_sb = {}    # g -> list of 4 x tiles
        q_sb = {}     # g -> list of 4 qT tiles
        exp_sb = {}   # g -> list of 8 exp tiles
        norm_sb = {}  # g -> normalized avT tile [128, 4*CHUNK]

        def dma_load(g):
            b, t0 = g // NCHUNK, (g % NCHUNK) * CHUNK
            tiles = []
            for k in range(KD):
                xk = xp.tile([128, CHUNK], bf16, tag=f"xt{k}", name=f"x{g}_{k}")
                nc.sync.dma_start(
                    out=xk, in_=xt[b, k * 128:(k + 1) * 128, t0:t0 + CHUNK]
                )
                tiles.append(xk)
            xt_sb[g] = tiles

        def qproj(g):
            tiles = []
            for f in range(KD):
                psq = pq.tile([128, CHUNK], f32, tag="q", name=f"psq{g}_{f}")
                for k in range(KD):
                    nc.tensor.matmul(
                        psq,
                        lhsT=wq_sb[k][:, f * 128:(f + 1) * 128],
                        rhs=xt_sb[g][k],
                        start=(k == 0),
                        stop=(k == KD - 1),
                    )
                qf = qp.tile([128, CHUNK], bf16, tag=f"q{f}", name=f"q{g}_{f}")
                nc.vector.tensor_copy(qf, psq)
                tiles.append(qf)
            del xt_sb[g]
            q_sb[g] = tiles

        # emit x-loads and qproj(0..1) before the K/V setup so the PE and the
        # DMA rings have work immediately instead of waiting on cond weights.
        dma_load(0)
        dma_load(1)

        # ---- setup-only weights live in a scoped pool, released after ----
        sp_setup = tc.alloc_tile_pool(name="sp_setup", bufs=1)
        wk_sb = load_rows(sp_setup, wkt, CDT, MODEL_DIM, "wk")
        wv_sb = load_rows(sp_setup, wvt, CDT, MODEL_DIM, "wv")
        cond_sb = []
        for k in range(CDT):
            t_ = sp_setup.tile([128, NB * LK], bf16, tag=f"cond{k}",
                               name=f"cond{k}")
            nc.sync.dma_start(out=t_, in_=condt[k * 128:(k + 1) * 128, :])
            cond_sb.append(t_)
        wo_sb = load_rows(wp, wot, KD, MODEL_DIM, "wo")

        qproj(0)
        dma_load(2)
        qproj(1)

        # ---- K projection (both batches at once): kT [512, NB*77] ----
        kt_sb = []
        for f in range(KD):
            psk = pq.tile([128, NB * LK], f32, tag="q", name=f"psk{f}")
            for c in range(CDT):
                nc.tensor.matmul(
                    psk,
                    lhsT=wk_sb[c][:, f * 128:(f + 1) * 128],
                    rhs=cond_sb[c],
                    start=(c == 0),
                    stop=(c == CDT - 1),
                )
            ktf = bp.tile([128, NB * LK], bf16, tag=f"kt{f}", name=f"kt{f}")
            nc.scalar.copy(ktf, psk)
            kt_sb.append(ktf)

        # ---- V projection per batch -> v_aug [77, 8*65] (65th col = ones) ----
        v_aug = []
        for b in range(NB):
            psv = pav.tile([LK, MODEL_DIM], f32, tag="av", name=f"psv{b}")
            for c in range(CDT):
                nc.tensor.matmul(
                    psv,
                    lhsT=cond_sb[c][:, b * LK:(b + 1) * LK],
                    rhs=wv_sb[c],
                    start=(c == 0),
                    stop=(c == CDT - 1),
                )
            va = bp.tile([LK, N_HEADS * (HEAD_DIM + 1)], bf16, tag=f"va{b}",
                         name=f"va{b}")
            for h in range(N_HEADS):
                nc.scalar.copy(
                    va[:, h * 65:h * 65 + 64], psv[:, h * 64:(h + 1) * 64]
                )
            ones_view = va.rearrange("p (h c) -> p h c", c=65)[:, :, 64]
            nc.vector.memset(ones_view, 1.0)
            v_aug.append(va)
        sp_setup.release()

        def scores_exp(g):
            b = g // NCHUNK
            tiles = []
            for p in range(N_HEADS // 2):
                for half in range(2):
                    h = 2 * p + half
                    lo, hi = 64 * half, 64 * (half + 1)
                    pss = ps.tile([LK, CHUNK], f32, tag="s", name=f"pss{g}_{h}")
                    nc.tensor.matmul(
                        pss,
                        lhsT=kt_sb[p][lo:hi, b * LK:(b + 1) * LK],
                        rhs=q_sb[g][p][lo:hi, :],
                        start=True,
                        stop=True,
                    )
                    e = epool.tile([LK, CHUNK], bf16, tag=f"e{h}", name=f"e{g}_{h}")
                    nc.scalar.activation(e, pss, Exp, scale=SCALE)
                    tiles.append(e)
            del q_sb[g]
            exp_sb[g] = tiles

        def av_norm(g):
            b = g // NCHUNK
            # attn @ V_aug per head; row 64 of each bank is the softmax
            # denominator.  Each bank is evacuated immediately (fast PSUM
            # release) into one of two [65, 4*CHUNK] bf16 tiles: avsb[h%2],
            # free slot h//2.  Denominator rows batch-gather with 2 DMAs; the
            # whole reciprocal chain (cast, approx-recip, bf16 cast) runs on
            # the Vector queue; a DRAM bounce + 2 stride-0 replicate DMAs
            # broadcast the reciprocals, and 2 wide bf16 multiplies produce
            # norm [128, 4*CHUNK] whose column-slices are the O-proj rhs.
            avsb = [
                avp.tile([HEAD_DIM + 1, KD * CHUNK], bf16, tag=f"av{i}",
                         name=f"avsb{g}_{i}")
                for i in range(2)
            ]
            for h in range(N_HEADS):
                pavt = pav.tile([HEAD_DIM + 1, CHUNK], f32, tag="av",
                                name=f"pav{g}_{h}")
                nc.tensor.matmul(
                    pavt,
                    lhsT=v_aug[b][:, h * 65:(h + 1) * 65],
                    rhs=exp_sb[g][h],
                    start=True,
                    stop=True,
                )
                dst = avsb[h % 2][:, (h // 2) * CHUNK:(h // 2 + 1) * CHUNK]
                if h % 2 == 0:
                    nc.scalar.copy(dst, pavt)
                else:
                    nc.vector.tensor_copy(dst, pavt)
            dt_ = rp.tile([N_HEADS, CHUNK], bf16, tag="dt", name=f"dt{g}")
            for i in range(2):
                nc.sync.dma_start(out=dt_[4 * i:4 * i + 4, :],
                                  in_=avsb[i][HEAD_DIM:HEAD_DIM + 1, :])
            df = rp.tile([N_HEADS, CHUNK], f32, tag="df", name=f"df{g}")
            nc.vector.tensor_copy(df, dt_)
            rf = rfp.tile([N_HEADS, CHUNK], f32, tag="rf", name=f"rf{g}")
            nc.vector.reciprocal_approx_fast(out=rf, in_=df)
            rb = rbp.tile([N_HEADS, CHUNK], bf16, tag="rb", name=f"rb{g}")
            nc.vector.tensor_copy(rb, rf)
            scr = drp.tile([N_HEADS, CHUNK], bf16, tag="scr", name=f"scr{g}")
            nc.sync.dma_start(out=scr, in_=rb)
            norm = npool.tile([128, KD * CHUNK], bf16, tag="n", name=f"n{g}")
            for i in range(2):
                bc = bcp.tile([HEAD_DIM, KD * CHUNK], bf16, tag=f"bc{i}",
                              name=f"bc{g}_{i}")
                row = scr[4 * i:4 * i + 1, :]
                if g >= NCTOT - 2:
                    # drain tail: split the replicate across 2 rings so the
                    # last O-projs aren't gated by a ~15us serial transfer
                    rep = bass.AP(row.tensor, row.offset,
                                  [[0, 32], [1, KD * CHUNK]])
                    nc.sync.dma_start(out=bc[0:32, :], in_=rep)
                    nc.sync.dma_start(out=bc[32:64, :], in_=rep)
                else:
                    rep = bass.AP(row.tensor, row.offset,
                                  [[0, HEAD_DIM], [1, KD * CHUNK]])
                    nc.sync.dma_start(out=bc, in_=rep)
                nc.vector.tensor_mul(
                    norm[64 * i:64 * (i + 1), :],
                    avsb[i][0:HEAD_DIM, :],
                    bc,
                )
            del exp_sb[g]
            norm_sb[g] = norm

        def oproj_store(g):
            b, t0 = g // NCHUNK, (g % NCHUNK) * CHUNK
            for f in range(KD):
                psy = py.tile([128, CHUNK], f32, tag="y", name=f"psy{g}_{f}")
                for j in range(KD):
                    nc.tensor.matmul(
                        psy,
                        lhsT=wo_sb[j][:, f * 128:(f + 1) * 128],
                        rhs=norm_sb[g][:, j * CHUNK:(j + 1) * CHUNK],
                        start=(j == 0),
                        stop=(j == KD - 1),
                    )
                ysb = yp.tile([128, CHUNK], bf16, tag=f"y{f}", name=f"y{g}_{f}")
                nc.scalar.copy(ysb, psy)
                nc.sync.dma_start(
                    out=yt[b, f * 128:(f + 1) * 128, t0:t0 + CHUNK], in_=ysb
                )
            del norm_sb[g]

        scores_exp(0)
        for i in range(1, NCTOT + 2):
            if i + 2 < NCTOT:
                dma_load(i + 2)
            if i + 1 < NCTOT:
                qproj(i + 1)
            if i < NCTOT:
                scores_exp(i)
            if 0 <= i - 2 < NCTOT:
                oproj_store(i - 2)
            if 0 <= i - 1 < NCTOT:
                av_norm(i - 1)

    nc.compile()
    return nc


def _get_program():
    global _PROG
    if _PROG is None:
        _PROG = _build_program()
    return _PROG


def _shard_inputs(x, cond, w_q, w_k, w_v, w_o):
    """Host-side layout: transpose + shard + bf16 cast. Returns per-core in_maps."""
    import ml_dtypes

    bf = ml_dtypes.bfloat16
    x = np.ascontiguousarray(x, dtype=bf)
    cond = np.ascontiguousarray(cond, dtype=bf)
    wqt = np.ascontiguousarray(w_q.T.astype(bf))
    wkt = np.ascontiguousarray(w_k.T.astype(bf))
    wvt = np.ascontiguousarray(w_v.T.astype(bf))
    wot = np.ascontiguousarray(w_o.T.astype(bf))

    xT = np.ascontiguousarray(x.transpose(0, 2, 1))          # [B, D, T]
    condT = np.ascontiguousarray(cond.transpose(0, 2, 1))    # [B, CD, LK]

    in_maps = []
    for c in range(N_CORES):
        b0 = c * NB
        ct = np.ascontiguousarray(
            condT[b0:b0 + NB].transpose(1, 0, 2).reshape(COND_DIM, NB * LK)
        )
        in_maps.append(
            {
                "xt": np.ascontiguousarray(xT[b0:b0 + NB]),
                "condt": ct,
                "wqt": wqt,
                "wkt": wkt,
                "wvt": wvt,
                "wot": wot,
            }
        )
    return in_maps


def kernel(x, cond, w_q, w_k, w_v, w_o):
    global LAST_RESULTS
    from concourse.bass_utils import run_bass_kernel_spmd

    nc = _get_program()
    in_maps = _shard_inputs(x, cond, w_q, w_k, w_v, w_o)
    trace = bool(os.environ.get("BASS_TRACE"))
    res = run_bass_kernel_spmd(
        nc, in_maps, list(range(N_CORES)), trace=trace
    )
    LAST_RESULTS = res

    out = np.empty((B, T, MODEL_DIM), dtype=np.float32)
    for c in range(N_CORES):
        ytc = np.asarray(res.results[c]["yt"], dtype=np.float32)   # [NB, D, T]
        out[c * NB:(c + 1) * NB] = ytc.transpose(0, 2, 1)
    return out



# revision 23
# speedup vs baseline: 1.0115x; 1.0115x over previous
"""Trainium2 Bass kernel: CrossAttnBlock (16x4096x512 query, 16x77x768 cond).

Sharding: pure data-parallel over batch -- 2 batches per core on 8 cores,
no collectives.  Host-side work is layout-only (transposes / slicing / bf16
cast).

On-device dataflow per core (activations kept transposed: feature dim on
SBUF partitions, tokens on the free dim; all matmul inputs bf16, PSUM f32):
    qT = wqT-stationary matmuls over xT chunks  [512f x 512t] per chunk
    kT = wkT-stationary matmuls over condT      [512f x 154s]
    v  = condT-stationary matmuls               [77s x 512d] -> v_aug [77, 8*65]
    scoresT_h = kT_h stationary @ qT_h          [77s x 512t]  (row-packed pairs)
    e_h = exp(scoresT_h / 8)                    (no max subtraction; scores ~ +-2)
    avT_h = v_aug_h @ e_h      [65, 512t] (row 64 = softmax denominator)
    denominator rows gathered by DMA into [8, 512], DVE cast + approx-
    reciprocal, DRAM bounce + stride-0 replicate DMAs broadcast the
    reciprocals, 2 wide bf16 DVE multiplies produce norm [128, 4*CHUNK]
    yT = woT-stationary matmuls over norm column-slices; bf16 evac + DMA out

The emission order is software-pipelined across chunks so each engine's
in-order stream has its cross-engine dependencies already satisfied:
    iter i:  scores+exp(i) | dma(i+2) | Qproj(i+1) | Oproj(i-3) | av+norm(i-1)
PSUM-evacuation copies are balanced across the Scalar and Vector queues;
the whole reciprocal chain stays on the Vector queue so the Scalar queue
never head-of-line blocks on it.  Score matmul head-pairs auto-derive
tile_position (0,0)/(64,0) and run concurrently on the PE's row groups.
"""

import os
import numpy as np

MODEL_DIM = 512
COND_DIM = 768
HEAD_DIM = 64
N_HEADS = 8
B = 16
T = 4096
LK = 77
N_CORES = 8
NB = B // N_CORES          # batches per core
CHUNK = 512                # tokens per chunk
NCHUNK = T // CHUNK
NCTOT = NB * NCHUNK        # total chunks per core
KD = MODEL_DIM // 128      # 4 partition tiles of model dim
CDT = COND_DIM // 128      # 6 partition tiles of cond dim
SCALE = HEAD_DIM ** -0.5

_PROG = None               # cached compiled Bass program
LAST_RESULTS = None        # BassKernelResults of last run (for profiling)


def _build_program():
    import concourse.bass as bass  # noqa: F401
    import concourse.tile as tile
    from concourse import bacc, mybir
    from contextlib import ExitStack

    f32 = mybir.dt.float32
    bf16 = mybir.dt.bfloat16
    Exp = mybir.ActivationFunctionType.Exp

    nc = bacc.Bacc(
        "TRN2", target_bir_lowering=False, debug=False, num_devices=N_CORES
    )

    xt = nc.dram_tensor("xt", [NB, MODEL_DIM, T], bf16, kind="ExternalInput").ap()
    condt = nc.dram_tensor(
        "condt", [COND_DIM, NB * LK], bf16, kind="ExternalInput"
    ).ap()
    wqt = nc.dram_tensor("wqt", [MODEL_DIM, MODEL_DIM], bf16, kind="ExternalInput").ap()
    wkt = nc.dram_tensor("wkt", [COND_DIM, MODEL_DIM], bf16, kind="ExternalInput").ap()
    wvt = nc.dram_tensor("wvt", [COND_DIM, MODEL_DIM], bf16, kind="ExternalInput").ap()
    wot = nc.dram_tensor("wot", [MODEL_DIM, MODEL_DIM], bf16, kind="ExternalInput").ap()
    yt = nc.dram_tensor("yt", [NB, MODEL_DIM, T], bf16, kind="ExternalOutput").ap()

    with tile.TileContext(nc) as tc, ExitStack() as ctx:
        wp = ctx.enter_context(tc.tile_pool(name="wp", bufs=1))
        bp = ctx.enter_context(tc.tile_pool(name="bp", bufs=1))   # per-batch stuff
        xp = ctx.enter_context(tc.tile_pool(name="xp", bufs=3))   # x chunks
        qp = ctx.enter_context(tc.tile_pool(name="qp", bufs=3))   # qT chunks
        epool = ctx.enter_context(tc.tile_pool(name="epool", bufs=3))
        avp = ctx.enter_context(tc.tile_pool(name="avp", bufs=3))  # evac'd attnV
        rp = ctx.enter_context(tc.tile_pool(name="rp", bufs=3))    # denom rows
        rfp = ctx.enter_context(tc.tile_pool(name="rfp", bufs=3))  # 1/denom f32
        rbp = ctx.enter_context(tc.tile_pool(name="rbp", bufs=3))  # 1/denom bf16
        bcp = ctx.enter_context(tc.tile_pool(name="bcp", bufs=2))  # broadcasts
        drp = ctx.enter_context(tc.tile_pool(name="drp", bufs=3, space="DRAM"))
        npool = ctx.enter_context(tc.tile_pool(name="npool", bufs=3))
        yp = ctx.enter_context(tc.tile_pool(name="yp", bufs=2))
        pq = ctx.enter_context(tc.tile_pool(name="pq", bufs=2, space="PSUM"))
        ps = ctx.enter_context(tc.tile_pool(name="ps", bufs=2, space="PSUM"))
        pav = ctx.enter_context(tc.tile_pool(name="pav", bufs=2, space="PSUM"))
        py = ctx.enter_context(tc.tile_pool(name="py", bufs=2, space="PSUM"))

        # ---- load weights ----
        def load_rows(pool, dram_ap, n_tiles, free, tagbase, split=1):
            tiles = []
            for k in range(n_tiles):
                t_ = pool.tile([128, free], bf16, tag=f"{tagbase}{k}",
                               name=f"{tagbase}{k}")
                step = 128 // split
                for s in range(split):
                    nc.sync.dma_start(
                        out=t_[s * step:(s + 1) * step, :],
                        in_=dram_ap[k * 128 + s * step:k * 128 + (s + 1) * step, :],
                    )
                tiles.append(t_)
            return tiles

        wq_sb = load_rows(wp, wqt, KD, MODEL_DIM, "wq")

        # ---- software-pipelined chunk stages --------------------------------
        # chunk g: batch b = g // NCHUNK, token offset t0 = (g % NCHUNK)*CHUNK
        xt_sb = {}    # g -> list of 4 x tiles
        q_sb = {}     # g -> list of 4 qT tiles
        exp_sb = {}   # g -> list of 8 exp tiles
        norm_sb = {}  # g -> normalized avT tile [128, 4*CHUNK]

        def dma_load(g):
            b, t0 = g // NCHUNK, (g % NCHUNK) * CHUNK
            tiles = []
            for k in range(KD):
                xk = xp.tile([128, CHUNK], bf16, tag=f"xt{k}", name=f"x{g}_{k}")
                nc.sync.dma_start(
                    out=xk, in_=xt[b, k * 128:(k + 1) * 128, t0:t0 + CHUNK]
                )
                tiles.append(xk)
            xt_sb[g] = tiles

        def qproj(g):
            tiles = []
            for f in range(KD):
                psq = pq.tile([128, CHUNK], f32, tag="q", name=f"psq{g}_{f}")
                for k in range(KD):
                    nc.tensor.matmul(
                        psq,
                        lhsT=wq_sb[k][:, f * 128:(f + 1) * 128],
                        rhs=xt_sb[g][k],
                        start=(k == 0),
                        stop=(k == KD - 1),
                    )
                qf = qp.tile([128, CHUNK], bf16, tag=f"q{f}", name=f"q{g}_{f}")
                nc.vector.tensor_copy(qf, psq)
                tiles.append(qf)
            del xt_sb[g]
            q_sb[g] = tiles

        # emit x-loads and qproj(0..1) before the K/V setup so the PE and the
        # DMA rings have work immediately instead of waiting on cond weights.
        dma_load(0)
        dma_load(1)

        # ---- setup-only weights live in a scoped pool, released after ----
        sp_setup = tc.alloc_tile_pool(name="sp_setup", bufs=1)
        wk_sb = load_rows(sp_setup, wkt, CDT, MODEL_DIM, "wk")
        wv_sb = load_rows(sp_setup, wvt, CDT, MODEL_DIM, "wv")
        cond_sb = []
        for k in range(CDT):
            t_ = sp_setup.tile([128, NB * LK], bf16, tag=f"cond{k}",
                               name=f"cond{k}")
            nc.sync.dma_start(out=t_, in_=condt[k * 128:(k + 1) * 128, :])
            cond_sb.append(t_)
        wo_sb = load_rows(wp, wot, KD, MODEL_DIM, "wo")

        qproj(0)
        dma_load(2)
        qproj(1)

        # ---- K projection (both batches at once): kT [512, NB*77] ----
        kt_sb = []
        for f in range(KD):
            psk = pq.tile([128, NB * LK], f32, tag="q", name=f"psk{f}")
            for c in range(CDT):
                nc.tensor.matmul(
                    psk,
                    lhsT=wk_sb[c][:, f * 128:(f + 1) * 128],
                    rhs=cond_sb[c],
                    start=(c == 0),
                    stop=(c == CDT - 1),
                )
            ktf = bp.tile([128, NB * LK], bf16, tag=f"kt{f}", name=f"kt{f}")
            nc.scalar.copy(ktf, psk)
            kt_sb.append(ktf)

        # ---- V projection per batch -> v_aug [77, 8*65] (65th col = ones) ----
        v_aug = []
        for b in range(NB):
            psv = pav.tile([LK, MODEL_DIM], f32, tag="av", name=f"psv{b}")
            for c in range(CDT):
                nc.tensor.matmul(
                    psv,
                    lhsT=cond_sb[c][:, b * LK:(b + 1) * LK],
                    rhs=wv_sb[c],
                    start=(c == 0),
                    stop=(c == CDT - 1),
                )
            va = bp.tile([LK, N_HEADS * (HEAD_DIM + 1)], bf16, tag=f"va{b}",
                         name=f"va{b}")
            for h in range(N_HEADS):
                nc.scalar.copy(
                    va[:, h * 65:h * 65 + 64], psv[:, h * 64:(h + 1) * 64]
                )
            ones_view = va.rearrange("p (h c) -> p h c", c=65)[:, :, 64]
            nc.vector.memset(ones_view, 1.0)
            v_aug.append(va)
        sp_setup.release()

        def scores_exp(g):
            b = g // NCHUNK
            tiles = []
            for p in range(N_HEADS // 2):
                for half in range(2):
                    h = 2 * p + half
                    lo, hi = 64 * half, 64 * (half + 1)
                    pss = ps.tile([LK, CHUNK], f32, tag="s", name=f"pss{g}_{h}")
                    nc.tensor.matmul(
                        pss,
                        lhsT=kt_sb[p][lo:hi, b * LK:(b + 1) * LK],
                        rhs=q_sb[g][p][lo:hi, :],
                        start=True,
                        stop=True,
                    )
                    e = epool.tile([LK, CHUNK], bf16, tag=f"e{h}", name=f"e{g}_{h}")
                    nc.scalar.activation(e, pss, Exp, scale=SCALE)
                    tiles.append(e)
            del q_sb[g]
            exp_sb[g] = tiles

        def av_norm(g):
            b = g // NCHUNK
            # attn @ V_aug per head; row 64 of each bank is the softmax
            # denominator.  Each bank is evacuated immediately (fast PSUM
            # release) into one of two [65, 4*CHUNK] bf16 tiles: avsb[h%2],
            # free slot h//2.  Denominator rows batch-gather with 2 DMAs; the
            # whole reciprocal chain (cast, approx-recip, bf16 cast) runs on
            # the Vector queue; a DRAM bounce + 2 stride-0 replicate DMAs
            # broadcast the reciprocals, and 2 wide bf16 multiplies produce
            # norm [128, 4*CHUNK] whose column-slices are the O-proj rhs.
            avsb = [
                avp.tile([HEAD_DIM + 1, KD * CHUNK], bf16, tag=f"av{i}",
                         name=f"avsb{g}_{i}")
                for i in range(2)
            ]
            for h in range(N_HEADS):
                pavt = pav.tile([HEAD_DIM + 1, CHUNK], f32, tag="av",
                                name=f"pav{g}_{h}")
                nc.tensor.matmul(
                    pavt,
                    lhsT=v_aug[b][:, h * 65:(h + 1) * 65],
                    rhs=exp_sb[g][h],
                    start=True,
                    stop=True,
                )
                dst = avsb[h % 2][:, (h // 2) * CHUNK:(h // 2 + 1) * CHUNK]
                if h % 2 == 0:
                    nc.scalar.copy(dst, pavt)
                else:
                    nc.vector.tensor_copy(dst, pavt)
            dt_ = rp.tile([N_HEADS, CHUNK], bf16, tag="dt", name=f"dt{g}")
            for i in range(2):
                nc.sync.dma_start(out=dt_[4 * i:4 * i + 4, :],
                                  in_=avsb[i][HEAD_DIM:HEAD_DIM + 1, :])
            df = rp.tile([N_HEADS, CHUNK], f32, tag="df", name=f"df{g}")
            nc.vector.tensor_copy(df, dt_)
            rf = rfp.tile([N_HEADS, CHUNK], f32, tag="rf", name=f"rf{g}")
            nc.vector.reciprocal_approx_fast(out=rf, in_=df)
            rb = rbp.tile([N_HEADS, CHUNK], bf16, tag="rb", name=f"rb{g}")
            nc.vector.tensor_copy(rb, rf)
            scr = drp.tile([N_HEADS, CHUNK], bf16, tag="scr", name=f"scr{g}")
            nc.sync.dma_start(out=scr, in_=rb)
            norm = npool.tile([128, KD * CHUNK], bf16, tag="n", name=f"n{g}")
            for i in range(2):
                bc = bcp.tile([HEAD_DIM, KD * CHUNK], bf16, tag=f"bc{i}",
                              name=f"bc{g}_{i}")
                row = scr[4 * i:4 * i + 1, :]
                rep = bass.AP(row.tensor, row.offset,
                              [[0, HEAD_DIM], [1, KD * CHUNK]])
                nc.sync.dma_start(out=bc, in_=rep)
                nc.vector.tensor_mul(
                    norm[64 * i:64 * (i + 1), :],
                    avsb[i][0:HEAD_DIM, :],
                    bc,
                )
            del exp_sb[g]
            norm_sb[g] = norm

        def oproj_store(g):
            b, t0 = g // NCHUNK, (g % NCHUNK) * CHUNK
            for f in range(KD):
                psy = py.tile([128, CHUNK], f32, tag="y", name=f"psy{g}_{f}")
                for j in range(KD):
                    nc.tensor.matmul(
                        psy,
                        lhsT=wo_sb[j][:, f * 128:(f + 1) * 128],
                        rhs=norm_sb[g][:, j * CHUNK:(j + 1) * CHUNK],
                        start=(j == 0),
                        stop=(j == KD - 1),
                    )
                ysb = yp.tile([128, CHUNK], bf16, tag=f"y{f}", name=f"y{g}_{f}")
                nc.scalar.copy(ysb, psy)
                nc.sync.dma_start(
                    out=yt[b, f * 128:(f + 1) * 128, t0:t0 + CHUNK], in_=ysb
                )
            del norm_sb[g]

        scores_exp(0)
        for i in range(1, NCTOT + 2):
            if i + 2 < NCTOT:
                dma_load(i + 2)
            if i + 1 < NCTOT:
                qproj(i + 1)
            if i < NCTOT:
                scores_exp(i)
            if 0 <= i - 2 < NCTOT:
                oproj_store(i - 2)
            if 0 <= i - 1 < NCTOT:
                av_norm(i - 1)

    nc.compile()
    return nc


def _get_program():
    global _PROG
    if _PROG is None:
        _PROG = _build_program()
    return _PROG


def _shard_inputs(x, cond, w_q, w_k, w_v, w_o):
    """Host-side layout: transpose + shard + bf16 cast. Returns per-core in_maps."""
    import ml_dtypes

    bf = ml_dtypes.bfloat16
    x = np.ascontiguousarray(x, dtype=bf)
    cond = np.ascontiguousarray(cond, dtype=bf)
    wqt = np.ascontiguousarray(w_q.T.astype(bf))
    wkt = np.ascontiguousarray(w_k.T.astype(bf))
    wvt = np.ascontiguousarray(w_v.T.astype(bf))
    wot = np.ascontiguousarray(w_o.T.astype(bf))

    xT = np.ascontiguousarray(x.transpose(0, 2, 1))          # [B, D, T]
    condT = np.ascontiguousarray(cond.transpose(0, 2, 1))    # [B, CD, LK]

    in_maps = []
    for c in range(N_CORES):
        b0 = c * NB
        ct = np.ascontiguousarray(
            condT[b0:b0 + NB].transpose(1, 0, 2).reshape(COND_DIM, NB * LK)
        )
        in_maps.append(
            {
                "xt": np.ascontiguousarray(xT[b0:b0 + NB]),
                "condt": ct,
                "wqt": wqt,
                "wkt": wkt,
                "wvt": wvt,
                "wot": wot,
            }
        )
    return in_maps


def kernel(x, cond, w_q, w_k, w_v, w_o):
    global LAST_RESULTS
    from concourse.bass_utils import run_bass_kernel_spmd

    nc = _get_program()
    in_maps = _shard_inputs(x, cond, w_q, w_k, w_v, w_o)
    trace = bool(os.environ.get("BASS_TRACE"))
    res = run_bass_kernel_spmd(
        nc, in_maps, list(range(N_CORES)), trace=trace
    )
    LAST_RESULTS = res

    out = np.empty((B, T, MODEL_DIM), dtype=np.float32)
    for c in range(N_CORES):
        ytc = np.asarray(res.results[c]["yt"], dtype=np.float32)   # [NB, D, T]
        out[c * NB:(c + 1) * NB] = ytc.transpose(0, 2, 1)
    return out

